# revision 1
# baseline (speedup 1.0000x reference)
"""Trainium2 Bass kernel v2: single-head causal attention with dropout.

reference:
    q,k,v = x@Wq, x@Wk, x@Wv          [B,T,H]
    wei = softmax(mask(q@k^T * H**-0.5))   (causal)
    wei = wei * (drop_u >= 0.2)/0.8
    out = wei @ v                      [B,T,H]

B=16, T=2048, D=1024, H=64. 8 NeuronCores, data-parallel over batch
(2 batches/core).

Design (evolved from the f32r baseline through traced iterations):
- bf16 datapath: x, W shipped bf16; q/k, E, P' stored bf16. u shipped
  fp8-e4m3 with threshold-aware rounding (round each element toward
  its side of 0.2) so the device comparison (u >= 0.2) matches the
  f32 reference EXACTLY. Halves/quarters DMA bytes and halves PE
  power vs f32r -- the PE sustains ~2.4GHz instead of ~1.25GHz.
- query-group PAIRS: scores psum [128, 1024] spanning 2 groups, ONE
  wide exp + ONE wide dropout-STT per key chunk (fewer fixed costs;
  engine op cost ~ free_size + fixed overhead).
- causal mask applied as a DVE 0/1-triangle multiply on E's diagonal
  block (cheaper than the identity@mask PE matmul, PE is the
  bottleneck engine).
- denominator accumulated into PSUM row 64 of the out^T bank via a
  [zeros|ones] stationary (no separate bank, no extra engine);
  1/d via ln/exp on ScalarE ([1,512] DVE reciprocal takes 3.4us --
  avoid); epilogue PE work deferred 2+ chunks so the PE never waits.
- wide causal u DMA: one [128 s, W] fp8 tile per (pair, key chunk),
  max-length lines (fewest DMA descriptors; queues are
  per-descriptor-overhead-bound at ~66ns/line).
- 1.25 dropout factor folded into Wv on host.
Engine budget per core (rested device): PE ~110-125us busy (the
bottleneck: ~68us streams at 2.4GHz + ~37us serialized LDWEIGHTS
[--enable-ldw-opt=false] + DVFS inflation), Scalar ~66, DVE ~60,
GpSimd ~2, DMA ~63us/queue.
"""

import numpy as np
from contextlib import ExitStack

import ml_dtypes

import concourse.bass as bass
import concourse.tile as tile
from concourse import mybir
from concourse.bass_utils import run_bass_kernel_spmd
from concourse.masks import make_identity

F32 = mybir.dt.float32
F32R = mybir.dt.float32r
BF16 = mybir.dt.bfloat16
F8 = mybir.dt.float8e4
BF = ml_dtypes.bfloat16
F8NP = ml_dtypes.float8_e4m3

B, T, D, H = 16, 2048, 1024, 64
N_CORES = 8
BPC = B // N_CORES
P_DROP = 0.2
NB = T // 128        # 16 key chunks per batch
NG = T // 512        # 4 query groups
PD = 2               # consume pipeline depth (in key chunks)
ULA = 2              # u DMA lookahead (in key chunks)


def _last_t(g):
    return 4 * g + 3


# walrus allows only ONE sync-wait per instruction; Tile can attach
# several. Move extras onto same-engine NOPs.
def _split_excess_waits(nc):
    n = 0
    for f in nc.m.functions:
        for bb in f.blocks:
            new_insts = []
            changed = False
            for inst in bb.instructions:
                si = inst.sync_info
                if si is not None and si.on_wait and len(si.on_wait) > 1:
                    waits = list(si.on_wait)
                    extra, keep = waits[:-1], waits[-1:]
                    for i, w in enumerate(extra):
                        new_insts.append(mybir.InstNoOp(
                            name=f"{inst.name}-ws-{i}",
                            engine=inst.engine, ins=[], outs=[],
                            sync_info=mybir.SyncInfo(on_wait=[w], on_update=[]),
                            text_hint="waitsplit", bass_nofuse=True))
                        n += 1
                    si.on_wait = keep
                    changed = True
                new_insts.append(inst)
            if changed:
                bb.instructions[:] = new_insts
    return n


def _build(ctx: ExitStack, tc: "tile.TileContext", xt, wqk, wv, ut, out):
    nc = tc.nc
    AF = mybir.ActivationFunctionType
    OP = mybir.AluOpType

    cpool = ctx.enter_context(tc.tile_pool(name="const", bufs=1))
    xpool = ctx.enter_context(tc.tile_pool(name="xt", bufs=2))
    qkvpool = ctx.enter_context(tc.tile_pool(name="qkv", bufs=2))
    upool = ctx.enter_context(tc.tile_pool(name="u", bufs=6))
    epool = ctx.enter_context(tc.tile_pool(name="e", bufs=5))
    pppool = ctx.enter_context(tc.tile_pool(name="pp", bufs=5))
    rdpool = ctx.enter_context(tc.tile_pool(name="rd", bufs=3))
    onpool = ctx.enter_context(tc.tile_pool(name="on", bufs=2))
    osbpool = ctx.enter_context(tc.tile_pool(name="osb", bufs=2))

    spool = ctx.enter_context(tc.tile_pool(name="sp", bufs=2, space="PSUM"))
    otps = ctx.enter_context(tc.tile_pool(name="ot", bufs=2, space="PSUM"))
    stageps = ctx.enter_context(tc.tile_pool(name="stage", bufs=2, space="PSUM"))

    # ---- constants -------------------------------------------------------
    identb = cpool.tile([128, 128], BF16)
    make_identity(nc, identb[:])

    # transposed block causal 0/1 mask: 1 where s <= q, 0 where s > q
    tri01 = cpool.tile([128, 128], BF16)
    nc.gpsimd.memset(tri01[:], 1.0)
    nc.gpsimd.affine_select(
        out=tri01[:], in_=tri01[:], compare_op=OP.is_ge, fill=0.0,
        base=0, pattern=[[1, 128]], channel_multiplier=-1)

    # denominator stationary: cols 0..63 zero, col 64 ones
    zo = cpool.tile([128, 65], BF16)
    nc.gpsimd.memset(zo[:], 0.0)
    nc.gpsimd.memset(zo[:, 64:65], 1.0)

    ones64f = cpool.tile([1, 64], F32)
    nc.gpsimd.memset(ones64f[:], 1.0)
    ones64 = cpool.tile([1, 64], F32R)
    nc.vector.tensor_copy(ones64[:], ones64f[:])

    wqk_sb = cpool.tile([128, 8 * 128], BF16)
    nc.sync.dma_start(
        wqk_sb[:].rearrange("p (c h) -> p c h", c=8),
        wqk.rearrange("(c p) h -> p c h", p=128))
    wv_sb = cpool.tile([128, 8 * H], BF16)
    nc.sync.dma_start(
        wv_sb[:].rearrange("p (c h) -> p c h", c=8),
        wv.rearrange("(c p) h -> p c h", p=128))

    xtiles = {}   # (b, half) -> list of 8 [128, 1024] tiles
    pending = []  # deferred epilogue finishes: [countdown, emit_fn]

    def flush_pending(force=False):
        while True:
            batch, pending[:] = pending[:], []
            rest = []
            for ent in batch:
                ent[0] -= 1
                if force or ent[0] <= 0:
                    ent[1]()   # may append new entries to `pending`
                else:
                    rest.append(ent)
            pending.extend(rest)
            if not force or not any(True for _ in pending):
                break
            if all(e[0] > 0 for e in pending) and not force:
                break
            if not pending:
                break

    def issue_xt(b, half, split=False):
        widths = (512, 512) if split else (1024,)
        col = 1024 * half
        for w in widths:
            tiles = []
            for c in range(8):
                xt_c = xpool.tile([128, 1024], BF16, tag=f"xt{c}",
                                  name=f"xt{c}")
                nc.sync.dma_start(
                    xt_c[:, 0:w],
                    xt[b, 128 * c:128 * (c + 1), col:col + w])
                tiles.append(xt_c)
            for q in range(col // 512, (col + w) // 512):
                xtiles[(b, q)] = (tiles, 512 * q - col)
            col += w

    def proj_quarter(b, Q, qkT, kT0, vTsb):
        xh, qoff = xtiles[(b, Q)]
        qkps = stageps.tile([128, 512], F32, tag="stage")
        vps = stageps.tile([64, 512], F32, tag="stage")
        # interleave qk/v matmuls so LDWEIGHTS hide under streams
        for c in range(8):
            nc.tensor.matmul(
                qkps[:], wqk_sb[:, 128 * c:128 * (c + 1)],
                xh[c][:, qoff:qoff + 512], start=(c == 0), stop=(c == 7))
            nc.tensor.matmul(
                vps[:], wv_sb[:, H * c:H * (c + 1)],
                xh[c][:, qoff:qoff + 512], start=(c == 0), stop=(c == 7))
        col = 512 * Q
        nc.scalar.copy(qkT[:, col:col + 512], qkps[:])
        nc.vector.tensor_copy(vTsb[:, col:col + 512], vps[:])
        # k^T rows 64..127 -> partitions 0..63 (for matmul base alignment)
        nc.sync.dma_start(kT0[:, col:col + 512], qkT[64:128, col:col + 512])

    def v_finalize(half, vTsb, v_sb):
        stg = stageps.tile([128, 512], BF16, tag="stage")
        for tloc in range(8):
            t = 8 * half + tloc
            nc.tensor.transpose(
                stg[:, 64 * tloc:64 * (tloc + 1)],
                vTsb[:, 128 * t:128 * (t + 1)], identb[:64, :64])
        nc.vector.tensor_copy(
            v_sb[:, 512 * half:512 * (half + 1)], stg[:])

    udicts = {}  # (b, P) -> {t: (u_tile, W)}

    def issue_u_for(b, P, t):
        us = udicts.setdefault((b, P), {})
        if t in us:
            return
        lo = max(1024 * P, 128 * t)
        W = 1024 * (P + 1) - lo
        u_t = upool.tile([128, 1024], F8, tag="u", name="u_t")
        nc.sync.dma_start(
            u_t[:, 0:W],
            ut[b, 128 * t:128 * (t + 1), lo:1024 * (P + 1)])
        us[t] = (u_t, W)

    def pair_loop(b, P, qkT, kT0, v_sb, weaves=None):
        tmax = 8 * (P + 1)
        glo = 2 * P
        ot = {g: otps.tile([65, 512], F32, tag="ot", name=f"ot{g}")
              for g in (glo, glo + 1)}
        us, es, pps = udicts.setdefault((b, P), {}), {}, {}

        def issue_u(t):
            issue_u_for(b, P, t)

        def produce(t):
            LO = max(0, 128 * t - 1024 * P)
            sp = spool.tile([128, 1024], F32, tag="sp")
            diag = None
            for g in (glo, glo + 1):
                qr = 128 * t - 512 * g
                if qr >= 512:
                    continue
                qo = max(0, qr)
                cs = 512 * (g - glo)
                if qr >= 0:
                    diag = cs + qo
                nc.tensor.matmul(
                    sp[:, cs + qo:cs + 512], kT0[:, 128 * t:128 * (t + 1)],
                    qkT[0:64, 512 * g + qo:512 * (g + 1)],
                    start=True, stop=True)
            E = epool.tile([128, 1024], BF16, tag="E")
            nc.scalar.activation(
                E[:, LO:1024], sp[:, LO:1024], AF.Exp, scale=float(H) ** -0.5)
            if diag is not None:
                nc.vector.tensor_mul(
                    E[:, diag:diag + 128], E[:, diag:diag + 128], tri01[:])
            u_t, W = us[t]
            Pp = pppool.tile([128, 1024], BF16, tag="Pp")
            nc.vector.scalar_tensor_tensor(
                Pp[:, LO:1024], u_t[:, 0:W], P_DROP, E[:, LO:1024],
                op0=OP.is_ge, op1=OP.mult)
            es[t] = (E, LO)
            pps[t] = Pp

        def consume(t):
            flush_pending()
            E, LO = es.pop(t)
            Pp = pps.pop(t)
            us.pop(t)
            gs = [g for g in (glo, glo + 1) if 128 * t - 512 * g < 512]
            for g in gs:
                qo = max(0, 128 * t - 512 * g)
                cs = 512 * (g - glo)
                nc.tensor.matmul(
                    ot[g][:, qo:512], zo[:], E[:, cs + qo:cs + 512],
                    start=(t == 0), stop=False, skip_group_check=True)
            for g in gs:
                qo = max(0, 128 * t - 512 * g)
                cs = 512 * (g - glo)
                nc.tensor.matmul(
                    ot[g][0:64, qo:512], v_sb[:, H * t:H * (t + 1)],
                    Pp[:, cs + qo:cs + 512],
                    start=False, stop=(t == _last_t(g)), skip_group_check=True)
            for g in gs:
                if t == _last_t(g):
                    epi_start(g, ot[g])

        def epi_start(g, otg):
            # 1/d as exp(-ln d) on ScalarE; PE-side finish is deferred so
            # the reciprocal latency hides under subsequent chunks. The
            # very last group of the kernel skips deferral and the xbar
            # transpose (nothing left to overlap; PE path is lower latency)
            last_group = (P == 1 and g == glo + 1 and b == BPC - 1)
            lnd = rdpool.tile([1, 512], F32, tag="lnd")
            nc.scalar.activation(lnd[:], otg[64:65, 0:512], AF.Ln)
            rd = rdpool.tile([1, 512], F32R, tag="rd")
            nc.scalar.activation(rd[:], lnd[:], AF.Exp, scale=-1.0)
            ot_sb = onpool.tile([64, 512], BF16, tag="otsb")
            nc.vector.tensor_copy(ot_sb[:], otg[0:64, :])
            if last_group:
                epi_finish(g, ot_sb, rd, now=True)
            else:
                pending.append([2, lambda: epi_finish(g, ot_sb, rd)])

        def epi_finish(g, ot_sb, rd, now=False):
            rb = stageps.tile([64, 512], F32, tag="stage")
            nc.tensor.matmul(rb[:], ones64[:], rd[:], start=True, stop=True)
            on_sb = onpool.tile([64, 512], BF16, tag="on")
            nc.vector.tensor_mul(on_sb[:], ot_sb[:], rb[:])
            if now:
                epi_finish2(g, on_sb, use_pe=True)
            else:
                pending.append([1, lambda: epi_finish2(g, on_sb)])

        def epi_finish2(g, on_sb, use_pe=False):
            if use_pe:
                onat = stageps.tile([128, 256], BF16, tag="stage")
                for cc in range(4):
                    nc.tensor.transpose(
                        onat[:, 64 * cc:64 * (cc + 1)],
                        on_sb[:, 128 * cc:128 * (cc + 1)], identb[:64, :64])
                onat_sb = onat
            else:
                onat_sb = onpool.tile([128, 256], BF16, tag="onat")
                nc.sync.dma_start_transpose(
                    onat_sb[:].rearrange("p (c h) -> p c h", c=4), on_sb[:])
            osb = osbpool.tile([128, 256], F32, tag="osb")
            nc.vector.tensor_copy(osb[:], onat_sb[:])
            nc.sync.dma_start(
                out[b].rearrange("(c p) h -> p c h", p=128)
                   [:, 4 * g:4 * (g + 1), :],
                osb[:].rearrange("p (c h) -> p c h", c=4))

        for i in range(ULA):
            issue_u(i)
        for t in range(tmax):
            if t + ULA < tmax:
                issue_u(t + ULA)
            for fn in (weaves or {}).get(t, []):
                fn()
            produce(t)
            if t >= PD:
                consume(t - PD)
        for t in range(max(0, tmax - PD), tmax):
            consume(t)

    tiles = {}

    def make_tiles(b):
        qkT = qkvpool.tile([128, T], BF16, tag="qkT", name="qkT")
        kT0 = qkvpool.tile([64, T], BF16, tag="kT0", name="kT0")
        vTsb = qkvpool.tile([64, T], BF16, tag="vT", name="vTsb")
        v_sb = qkvpool.tile([128, NB * H], BF16, tag="vsb", name="v_sb")
        tiles[b] = (qkT, kT0, vTsb, v_sb)
        return tiles[b]

    # batch-0 prologue: first half of phase A runs unoverlapped
    qkT, kT0, vTsb, v_sb = make_tiles(0)
    issue_xt(0, 0, split=True)
    proj_quarter(0, 0, qkT, kT0, vTsb)
    proj_quarter(0, 1, qkT, kT0, vTsb)
    v_finalize(0, vTsb, v_sb)
    issue_xt(0, 1)
    for b in range(BPC):
        qkT, kT0, vTsb, v_sb = tiles[b]
        pair_loop(b, 0, qkT, kT0, v_sb)
        for i in range(ULA):
            issue_u_for(b, 1, i)
        proj_quarter(b, 2, qkT, kT0, vTsb)
        proj_quarter(b, 3, qkT, kT0, vTsb)
        v_finalize(1, vTsb, v_sb)
        w1 = {}
        if b + 1 < BPC:
            nqkT, nkT0, nvTsb, nv_sb = make_tiles(b + 1)
            w1 = {1: [lambda nb=b + 1: issue_xt(nb, 0)],
                  11: [lambda nb=b + 1: issue_xt(nb, 1)]}
        pair_loop(b, 1, qkT, kT0, v_sb, weaves=w1)
        if b + 1 < BPC:
            nqkT, nkT0, nvTsb, nv_sb = tiles[b + 1]
            for i in range(ULA):
                issue_u_for(b + 1, 0, i)
            proj_quarter(b + 1, 0, nqkT, nkT0, nvTsb)
            proj_quarter(b + 1, 1, nqkT, nkT0, nvTsb)
            v_finalize(0, nvTsb, nv_sb)
    flush_pending(force=True)


_CACHE = {}


def _get_nc():
    if "nc" not in _CACHE:
        nc = bass.Bass("TRN2", target_bir_lowering=False)
        xt = nc.dram_tensor("xt", [BPC, D, T], BF16, kind="ExternalInput")
        wqk = nc.dram_tensor("wqk", [D, 128], BF16, kind="ExternalInput")
        wv = nc.dram_tensor("wv", [D, H], BF16, kind="ExternalInput")
        ut = nc.dram_tensor("ut", [BPC, T, T], F8, kind="ExternalInput")
        out = nc.dram_tensor("out", [BPC, T, H], F32, kind="ExternalOutput")
        with tile.TileContext(nc) as tc:
            with ExitStack() as ctx:
                _build(ctx, tc, xt.ap(), wqk.ap(), wv.ap(), ut.ap(), out.ap())
        _split_excess_waits(nc)
        _CACHE["nc"] = nc
    return _CACHE["nc"]


def _u_f8_exact(u):
    """fp8-e4m3 cast of u that preserves (u >= 0.2) exactly: round each
    element toward the side of the threshold it is on."""
    ub = u.astype(F8NP)
    hi_b = F8NP(0.203125)   # smallest e4m3 >= 0.2
    lo_b = F8NP(0.1875)     # largest e4m3 < 0.2
    assert float(hi_b) >= P_DROP > float(lo_b)
    ge = u >= np.float32(P_DROP)
    return np.where(ge, np.maximum(ub, hi_b), np.minimum(ub, lo_b)).astype(F8NP)


def kernel(x, Wq, Wk, Wv, drop_u, _trace=False):
    x = np.asarray(x, dtype=np.float32)
    Wq = np.asarray(Wq, dtype=np.float32)
    Wk = np.asarray(Wk, dtype=np.float32)
    Wv = np.asarray(Wv, dtype=np.float32)
    drop_u = np.asarray(drop_u, dtype=np.float32)

    nc = _get_nc()
    xb = x.astype(BF)
    xtf = np.ascontiguousarray(xb.transpose(0, 2, 1))          # [B, D, T]
    ub = _u_f8_exact(drop_u)
    utf = np.ascontiguousarray(ub.transpose(0, 2, 1))          # [B, s, q]
    wqk = np.ascontiguousarray(
        np.concatenate([Wq, Wk], axis=1)).astype(BF)           # [D, 128]
    wv15 = (Wv * np.float32(1.0 / (1.0 - P_DROP))).astype(BF)  # [D, 64]
    in_maps = []
    for c in range(N_CORES):
        lo = BPC * c
        in_maps.append({
            "xt": xtf[lo:lo + BPC],
            "wqk": wqk, "wv": wv15,
            "ut": utf[lo:lo + BPC],
        })
    res = run_bass_kernel_spmd(
        nc, in_maps, core_ids=list(range(N_CORES)), trace=_trace)
    outv = np.concatenate(
        [res.results[c]["out"] for c in range(N_CORES)], axis=0)
    if _trace:
        kernel.last_exec_time_ns = res.exec_time_ns
        kernel.last_results = res
    return outv

